# revision 1
# baseline (speedup 1.0000x reference)
"""AGNN (3-layer) Trainium2 Bass kernel, 8-core SPMD.

Sharding: dst-node shards (8192 padded rows/core). Per layer:
  normalize own shard -> bf16 TAB rows [xn|xu] -> AllGather full TAB ->
  per degree-class-group: dma_gather 512B rows (edge-major, k-major slots) ->
  DVE dot-trees -> segment softmax (fixed-K segments along free dim) ->
  weighted sum via DVE k-trees -> layer output in SBUF.
Sources split across two 32768-row table halves so gather indices fit int16.
"""

import numpy as np
import sys, os
from contextlib import ExitStack

for _p in ("/opt/trn_rl_repo", "/root/.axon_site/_ro/trn_rl_repo"):
    if os.path.isdir(_p) and _p not in sys.path:
        try:
            import concourse  # noqa
            break
        except Exception:
            sys.path.insert(0, _p)

NCORE = 8
N = 50000
D = 128
NSH_REAL = 6250
NSH = 8192  # 64*128; 4*NSH = 32768 rows per table half -> idx fits int16
NTAB = NCORE * NSH
A_CORES = 4
KCLASSES = (1, 2, 4, 8, 16, 32)
CHUNK_SLOTS = 4096


def _next_class(d):
    for k in KCLASSES:
        if d <= k:
            return k
    raise AssertionError(f"degree {d} exceeds max class 32")


def _build_structures(edge_index):
    src_g = edge_index[0].astype(np.int64)
    dst_g = edge_index[1].astype(np.int64)
    loops = np.arange(N, dtype=np.int64)
    src_g = np.concatenate([src_g, loops])
    dst_g = np.concatenate([dst_g, loops])
    core_of = np.minimum(dst_g // NSH_REAL, NCORE - 1)
    src_core = np.minimum(src_g // NSH_REAL, NCORE - 1)
    src_is_A = src_core < A_CORES
    B_BASE = A_CORES * NSH

    core_data = []
    for c in range(NCORE):
        lo = c * NSH_REAL
        hi = min((c + 1) * NSH_REAL, N)
        nreal = hi - lo
        emask = core_of == c
        e_src = src_g[emask]
        e_dst_loc = dst_g[emask] - lo
        e_isA = src_is_A[emask]
        dA = np.bincount(e_dst_loc[e_isA], minlength=nreal)
        dB = np.bincount(e_dst_loc[~e_isA], minlength=nreal)
        assert dA.max() <= 32 and dB.max() <= 32
        KA = np.maximum(np.array([_next_class(d) for d in dA]), 1)
        KB = np.array([_next_class(max(d, 1)) for d in dB])
        core_data.append(dict(c=c, lo=lo, nreal=nreal, e_src=e_src,
                              e_dst_loc=e_dst_loc, e_isA=e_isA, KA=KA, KB=KB))

    frozen = set()
    while True:
        allp = np.concatenate([np.stack([cd["KA"], cd["KB"]], 1) for cd in core_data])
        pairs, counts = np.unique(allp, axis=0, return_counts=True)
        tot_padded = 0
        for ka, kb in pairs:
            mx = max(int(((cd["KA"] == ka) & (cd["KB"] == kb)).sum()) for cd in core_data)
            tot_padded += (mx + 127) // 128 * 128
        fits = tot_padded <= NSH
        mergeable = np.array([
            ((counts[i] < 8 * 160) or not fits)
            and (tuple(pairs[i]) not in frozen)
            and (pairs[i][0] < 32 or pairs[i][1] < 32)
            for i in range(len(pairs))
        ])
        if not mergeable.any() or (len(pairs) <= 12 and fits):
            break
        gi = np.argmin(np.where(mergeable, counts, np.inf))
        ka, kb = pairs[gi]
        for cd in core_data:
            m = (cd["KA"] == ka) & (cd["KB"] == kb)
            if (ka <= kb or kb >= 32) and ka < 32:
                cd["KA"][m] = _next_class(ka + 1)
            elif kb < 32:
                cd["KB"][m] = _next_class(kb + 1)
            else:
                frozen.add((int(ka), int(kb)))
                break

    allp = np.concatenate([np.stack([cd["KA"], cd["KB"]], 1) for cd in core_data])
    pairs = np.unique(allp, axis=0)
    gsizes = []
    for ka, kb in pairs:
        mx = max(int(((cd["KA"] == ka) & (cd["KB"] == kb)).sum()) for cd in core_data)
        gsizes.append((mx + 127) // 128 * 128)
    assert sum(gsizes) <= NSH, f"{sum(gsizes)} > {NSH}"

    per_core = []
    for cd in core_data:
        groups, perm = [], []
        for (ka, kb), gn in zip(pairs, gsizes):
            nodes = np.nonzero((cd["KA"] == ka) & (cd["KB"] == kb))[0]
            ids = np.concatenate([nodes, -np.ones(gn - len(nodes), dtype=np.int64)])
            groups.append((int(ka), int(kb), gn))
            perm.append(ids)
        per_core.append(dict(c=cd["c"], lo=cd["lo"], nreal=cd["nreal"],
                             perm=np.concatenate(perm), groups=groups,
                             e_src=cd["e_src"], e_dst_loc=cd["e_dst_loc"],
                             e_isA=cd["e_isA"], A_BASE=0, B_BASE=B_BASE))

    perm_local = np.full(N, -1, dtype=np.int64)
    for st in per_core:
        pos = np.nonzero(st["perm"] >= 0)[0]
        perm_local[st["lo"] + st["perm"][pos]] = pos
    assert (perm_local >= 0).all()
    tabid = np.minimum(np.arange(N) // NSH_REAL, NCORE - 1) * NSH + perm_local

    for st in per_core:
        e_tab = tabid[st["e_src"]]
        eA, eB = {}, {}
        for s, d, isA in zip(e_tab, st["e_dst_loc"], st["e_isA"]):
            (eA if isA else eB).setdefault(d, []).append(s)
        gslots = []
        node_base = 0
        for ka, kb, gn in st["groups"]:
            SA, SB = gn * ka, gn * kb
            idxA = np.zeros(SA, dtype=np.int64)
            idxB = np.zeros(SB, dtype=np.int64)
            maskA = np.full(SA, -1e30, dtype=np.float32)
            maskB = np.full(SB, -1e30, dtype=np.float32)
            for j in range(gn):
                nid = st["perm"][node_base + j]
                blk, m = j // 128, j % 128
                if nid >= 0:
                    for k, s in enumerate(eA.get(nid, [])):
                        i = (blk * ka + k) * 128 + m
                        idxA[i] = s
                        maskA[i] = 0.0
                    for k, s in enumerate(eB.get(nid, [])):
                        i = (blk * kb + k) * 128 + m
                        idxB[i] = s - st["B_BASE"]
                        maskB[i] = 0.0
            gslots.append(dict(ka=ka, kb=kb, gn=gn, idxA=idxA, idxB=idxB,
                               maskA=maskA, maskB=maskB))
            node_base += gn
        st["gslots"] = gslots
        st["used_nodes"] = node_base
    return per_core, tabid


def _wrap_idx16(idx):
    S = len(idx)
    w = idx.reshape(S // 16, 16).T.astype(np.int16)
    return np.tile(w, (8, 1))


def _build_core_inputs(per_core):
    out = []
    for st in per_core:
        callsA, callsB, maskAc, maskBc, callplan = [], [], [], [], []
        node_off = 0
        for g in st["gslots"]:
            ka, kb, gn = g["ka"], g["kb"], g["gn"]
            kmax = max(ka, kb)
            npc = max(CHUNK_SLOTS // kmax // 128 * 128, 128)
            for nb in range(0, gn, npc):
                nn = min(npc, gn - nb)
                b0, b1 = nb // 128, (nb + nn) // 128
                callplan.append(dict(
                    ka=ka, kb=kb, node_off=node_off + nb, nodes=nn,
                    colsA=(b1 - b0) * ka, colsB=(b1 - b0) * kb,
                    offA=sum(len(x) for x in callsA) // 16,
                    offB=sum(len(x) for x in callsB) // 16,
                    moffA=sum(len(x) for x in maskAc) // 128,
                    moffB=sum(len(x) for x in maskBc) // 128,
                ))
                callsA.append(g["idxA"][b0 * ka * 128: b1 * ka * 128])
                callsB.append(g["idxB"][b0 * kb * 128: b1 * kb * 128])
                maskAc.append(g["maskA"][b0 * ka * 128: b1 * ka * 128])
                maskBc.append(g["maskB"][b0 * kb * 128: b1 * kb * 128])
            node_off += gn
        idxA = np.concatenate([_wrap_idx16(s) for s in callsA], axis=1)
        idxB = np.concatenate([_wrap_idx16(s) for s in callsB], axis=1)
        mA = np.concatenate(maskAc).reshape(-1, 128).T.copy().astype(np.float32)
        mB = np.concatenate(maskBc).reshape(-1, 128).T.copy().astype(np.float32)
        out.append(dict(idxA=idxA, idxB=idxB, maskA=mA, maskB=mB, callplan=callplan))
    return out


def _build_nc(plan, WA, WB, CA, CB, nlayers=3, ncalls=None, stage=4):
    import concourse.bass as bass
    import concourse.bacc as bacc
    import concourse.tile as tile
    from concourse import mybir, library_config

    f32, bf, i16 = mybir.dt.float32, mybir.dt.bfloat16, mybir.dt.int16
    Alu = mybir.AluOpType
    Act = mybir.ActivationFunctionType
    NB = NSH // 128

    nc = bacc.Bacc("TRN2", target_bir_lowering=False, debug=False, num_devices=NCORE, num_swdge_queues=2)
    x_in = nc.dram_tensor("x_shard", [NSH, D], f32, kind="ExternalInput")
    idxA_d = nc.dram_tensor("idxA", [128, WA], i16, kind="ExternalInput")
    idxB_d = nc.dram_tensor("idxB", [128, WB], i16, kind="ExternalInput")
    maskA_d = nc.dram_tensor("maskA", [128, CA], f32, kind="ExternalInput")
    maskB_d = nc.dram_tensor("maskB", [128, CB], f32, kind="ExternalInput")
    beta_d = nc.dram_tensor("betas", [1, 4], f32, kind="ExternalInput")
    y_d = nc.dram_tensor("y", [NSH, D], f32, kind="ExternalOutput")

    with ExitStack() as ctx:
        tc = ctx.enter_context(tile.TileContext(nc))
        res = ctx.enter_context(tc.tile_pool(name="res", bufs=1))
        dram = ctx.enter_context(tc.tile_pool(name="dram", bufs=1, space="DRAM"))
        gat = ctx.enter_context(tc.tile_pool(name="gat", bufs=2))
        sc = ctx.enter_context(tc.tile_pool(name="sc", bufs=1))
        vp = ctx.enter_context(tc.tile_pool(name="vp", bufs=2))
        nrm = ctx.enter_context(tc.tile_pool(name="nrm", bufs=3))
        sm = ctx.enter_context(tc.tile_pool(name="sm", bufs=1))
        ocp = ctx.enter_context(tc.tile_pool(name="ocp", bufs=2))

        xn_bf = res.tile([128, NB, D], bf)
        xu_bf = res.tile([128, NB, D], bf)
        idxA_s = res.tile([128, WA], i16)
        idxB_s = res.tile([128, WB], i16)
        maskA_s = res.tile([128, CA], f32)
        maskB_s = res.tile([128, CB], f32)
        beta_s = res.tile([128, 4], f32)
        eps_s = res.tile([128, 1], f32)
        tab_shard = dram.tile([NSH, 2 * D], bf)
        tab_full = dram.tile([NTAB, 2 * D], bf)
        xdr = dram.tile([NSH, D], f32)

        nc.gpsimd.load_library(library_config.mlp)
        nc.sync.dma_start(out=idxA_s[:], in_=idxA_d[:])
        nc.sync.dma_start(out=idxB_s[:], in_=idxB_d[:])
        nc.sync.dma_start(out=maskA_s[:], in_=maskA_d[:])
        nc.sync.dma_start(out=maskB_s[:], in_=maskB_d[:])
        bap = beta_d[:]
        nc.gpsimd.dma_start(
            out=beta_s[:],
            in_=bass.AP(tensor=bap.tensor, offset=bap.offset, ap=[[0, 128], [1, 4]]),
        )
        nc.vector.memset(eps_s[:], 1e-20)

        _regcache = {}

        def nreg(v):
            if v not in _regcache:
                _regcache[v] = nc.gpsimd.to_reg(v)
            return _regcache[v]

        def bcast_mid(ap3, k):
            a = ap3.ap
            return bass.AP(tensor=ap3.tensor, offset=ap3.offset,
                           ap=[a[0], a[1], [0, k], a[2]])

        def bcast_last(ap2, k):
            a = ap2.ap
            return bass.AP(tensor=ap2.tensor, offset=ap2.offset,
                           ap=[a[0], a[1], [0, k]])

        maxnblk = max(c["nodes"] // 128 for c in plan)

        for layer in range(nlayers):
            src = x_in if layer == 0 else xdr
            dst = y_d if layer == nlayers - 1 else xdr
            srcv = src[:].rearrange("(b m) d -> m b d", m=128)
            dstv = dst[:].rearrange("(b m) d -> m b d", m=128)

            # ---- normalize own shard (streamed), build TAB shard ----
            sq = sm.tile([128, NB], f32, tag="sq")
            for bb in range(0, NB, 8):
                xsb = nrm.tile([128, 8, D], f32, tag="xsb")
                nc.sync.dma_start(out=xsb[:], in_=srcv[:, bb : bb + 8, :])
                junk = sc.tile([128, 8, D], f32, tag="junk")
                nc.vector.tensor_mul(out=junk[:], in0=xsb[:], in1=xsb[:])
                nc.vector.tensor_reduce(
                    out=sq[:, bb : bb + 8],
                    in_=junk[:],
                    axis=mybir.AxisListType.X, op=Alu.add,
                )
                rstd8 = nrm.tile([128, 8], f32, tag="rstd8")
                nc.scalar.activation(out=rstd8[:], in_=sq[:, bb : bb + 8],
                                     func=Act.Sqrt, bias=eps_s[:], scale=1.0)
                nc.vector.reciprocal(out=rstd8[:], in_=rstd8[:])
                for j in range(8):
                    b = bb + j
                    nc.vector.tensor_scalar_mul(
                        out=xn_bf[:, b, :], in0=xsb[:, j, :],
                        scalar1=rstd8[:, j : j + 1],
                    )
                nc.vector.tensor_copy(out=xu_bf[:, bb : bb + 8, :], in_=xsb[:])
            tsv = tab_shard[:].rearrange("(b m) d -> m b d", m=128)
            nc.sync.dma_start(out=tsv[:, :, 0:D], in_=xn_bf[:])
            nc.sync.dma_start(out=tsv[:, :, D : 2 * D], in_=xu_bf[:])
            nc.gpsimd.collective_compute(
                "AllGather", Alu.bypass, replica_groups=[list(range(NCORE))],
                ins=[tab_shard[:]], outs=[tab_full[:]],
            )

            # ---- message passing ----
            for call in (plan if ncalls is None else plan[:ncalls]):
                ka, kb = call["ka"], call["kb"]
                colsA, colsB = call["colsA"], call["colsB"]
                nblk = call["nodes"] // 128
                b0 = call["node_off"] // 128

                def bucket(kx, colsX, offX, moffX, idx_s, mask_s, base_row, gtag):
                    tfull = gat.tile([128, 32, 2 * D], bf, tag=gtag)
                    t = tfull[:, 0:colsX, :]
                    for sc0 in range(0, colsX, 8):
                        scw = min(8, colsX - sc0)
                        nc.gpsimd.dma_gather(
                            out_ap=tfull[:, sc0 : sc0 + scw, :],
                            in_ap=tab_full[base_row : base_row + A_CORES * NSH, :],
                            idxs_ap=idx_s[:, offX + sc0 * 8 : offX + (sc0 + scw) * 8],
                            num_idxs=scw * 128,
                            num_idxs_reg=nreg(scw * 128),
                            elem_size=2 * D,
                            queue_num=(sc0 // 8) % 2,
                        )
                    if stage <= 1:
                        return tfull, None, None
                    Pf = sc.tile([128, 32, D], bf, tag="P")
                    P = Pf[:, 0:colsX, :]
                    xnsl = xn_bf[:, b0 : b0 + nblk, :]
                    nc.vector.tensor_tensor(
                        out=P.rearrange("p (b k) d -> p b k d", k=kx),
                        in0=t[:, :, 0:D].rearrange("p (b k) d -> p b k d", k=kx),
                        in1=bcast_mid(xnsl, kx),
                        op=Alu.mult,
                    )
                    cur = P
                    h = D // 2
                    while h >= 1:
                        dt = bf if h >= 16 else f32
                        nxt = sc.tile([128, 32, h], dt, tag=f"t{h}", name=f"t{h}")[:, 0:colsX, :]
                        nc.vector.tensor_add(
                            out=nxt, in0=cur[:, :, 0:h], in1=cur[:, :, h : 2 * h]
                        )
                        cur = nxt
                        h //= 2
                    alpha = cur.rearrange("p c one -> p (c one)")
                    am = sm.tile([128, 32], f32, tag="am" + gtag, name="am")[:, 0:colsX]
                    nc.vector.scalar_tensor_tensor(
                        out=am, in0=alpha, scalar=beta_s[:, layer : layer + 1],
                        in1=mask_s[:, moffX : moffX + colsX],
                        op0=Alu.mult, op1=Alu.add,
                    )
                    MX = sm.tile([128, 32], f32, tag="M" + gtag, name="MX")[:, 0:nblk]
                    nc.vector.tensor_reduce(
                        out=MX, in_=am.rearrange("p (b k) -> p b k", k=kx),
                        axis=mybir.AxisListType.X, op=Alu.max,
                    )
                    return tfull, am, MX

                tA, amA, MA = bucket(ka, colsA, call["offA"], call["moffA"],
                                     idxA_s, maskA_s, 0, "gA")
                tB, amB, MB = bucket(kb, colsB, call["offB"], call["moffB"],
                                     idxB_s, maskB_s, A_CORES * NSH, "gB")
                if stage <= 1:
                    oc = ocp.tile([128, maxnblk, D], f32, tag="oc", name="oc")[:, 0:nblk, :]
                    nc.vector.tensor_copy(out=oc, in_=tA[:, 0:nblk, 0:D])
                    nc.sync.dma_start(out=dstv[:, b0 : b0 + nblk, :], in_=oc)
                    continue

                M = sm.tile([128, 32], f32, tag="Mc", name="Mc")[:, 0:nblk]
                nc.vector.tensor_tensor(out=M, in0=MA, in1=MB, op=Alu.max)
                if stage <= 2:
                    oc = ocp.tile([128, maxnblk, D], f32, tag="oc", name="oc")[:, 0:nblk, :]
                    nc.vector.tensor_copy(out=oc, in_=bcast_mid(M[:, :, None] if False else bass.AP(tensor=M.tensor, offset=M.offset, ap=[M.ap[0], M.ap[1], [0, D]]), 1).rearrange("p b one d -> p (b one) d") if False else bass.AP(tensor=M.tensor, offset=M.offset, ap=[M.ap[0], M.ap[1], [0, D]]))
                    nc.sync.dma_start(out=dstv[:, b0 : b0 + nblk, :], in_=oc)
                    continue

                def softmax_part(am, kx, colsX, gtag):
                    E = sm.tile([128, 32], f32, tag="E" + gtag, name="E")[:, 0:colsX]
                    nc.vector.tensor_tensor(
                        out=E.rearrange("p (b k) -> p b k", k=kx),
                        in0=am.rearrange("p (b k) -> p b k", k=kx),
                        in1=bcast_last(M, kx),
                        op=Alu.subtract,
                    )
                    nc.scalar.activation(out=E, in_=E, func=Act.Exp)
                    ZX = sm.tile([128, 32], f32, tag="Z" + gtag, name="ZX")[:, 0:nblk]
                    nc.vector.tensor_reduce(
                        out=ZX, in_=E.rearrange("p (b k) -> p b k", k=kx),
                        axis=mybir.AxisListType.X, op=Alu.add,
                    )
                    return E, ZX

                EA, ZA = softmax_part(amA, ka, colsA, "gA")
                EB, ZB = softmax_part(amB, kb, colsB, "gB")
                Z = sm.tile([128, 32], f32, tag="Zc", name="Zc")[:, 0:nblk]
                nc.vector.scalar_tensor_tensor(
                    out=Z, in0=ZA, scalar=1e-30, in1=ZB,
                    op0=Alu.add, op1=Alu.add,
                )
                Zi = sm.tile([128, 32], f32, tag="Zi", name="Zic")[:, 0:nblk]
                nc.vector.reciprocal(out=Zi, in_=Z)
                if stage <= 3:
                    oc = ocp.tile([128, maxnblk, D], f32, tag="oc", name="oc")[:, 0:nblk, :]
                    nc.vector.tensor_copy(out=oc, in_=bass.AP(tensor=Zi.tensor, offset=Zi.offset, ap=[Zi.ap[0], Zi.ap[1], [0, D]]))
                    nc.sync.dma_start(out=dstv[:, b0 : b0 + nblk, :], in_=oc)
                    continue

                def agg_part(E, t, kx, colsX, gtag):
                    w = sm.tile([128, 32], bf, tag="w" + gtag, name="w")[:, 0:colsX]
                    nc.vector.tensor_tensor(
                        out=w.rearrange("p (b k) -> p b k", k=kx),
                        in0=E.rearrange("p (b k) -> p b k", k=kx),
                        in1=bcast_last(Zi, kx),
                        op=Alu.mult,
                    )
                    V = vp.tile([128, 32, D], bf, tag="V", name="V")[:, 0:colsX, :]
                    nc.vector.tensor_tensor(
                        out=V, in0=t[:, 0:colsX, D : 2 * D],
                        in1=bcast_last(w, D), op=Alu.mult,
                    )
                    cur = V.rearrange("p (b k) d -> p b k d", k=kx)
                    h = kx // 2
                    while h >= 1:
                        dt = bf if h > 1 else f32
                        vtag = f"v{h}{gtag}" if h == 1 else f"v{h}"
                        nxt = sc.tile([128, 16, 1, D], dt, tag=vtag, name=f"v{h}")
                        nxt = nxt[:].rearrange("p a one d -> p (a one) d")[
                            :, 0 : nblk * h, :
                        ].rearrange("p (b k) d -> p b k d", k=h)
                        nc.vector.tensor_add(
                            out=nxt, in0=cur[:, :, 0:h, :], in1=cur[:, :, h : 2 * h, :]
                        )
                        cur = nxt
                        h //= 2
                    return cur

                oA = agg_part(EA, tA, ka, colsA, "gA")
                oB = agg_part(EB, tB, kb, colsB, "gB")
                oc = ocp.tile([128, maxnblk, D], f32, tag="oc", name="oc")[:, 0:nblk, :]
                nc.vector.tensor_add(
                    out=oc,
                    in0=oA.rearrange("p b one d -> p (b one) d"),
                    in1=oB.rearrange("p b one d -> p (b one) d"),
                )
                nc.sync.dma_start(out=dstv[:, b0 : b0 + nblk, :], in_=oc)
    nc.compile()
    return nc


def kernel(x, edge_index, beta1, beta2, beta3, trace=False, _ret_info=None):
    x = np.asarray(x, dtype=np.float32)
    edge_index = np.asarray(edge_index)
    per_core, tabid = _build_structures(edge_index)
    core_inputs = _build_core_inputs(per_core)
    WA = core_inputs[0]["idxA"].shape[1]
    WB = core_inputs[0]["idxB"].shape[1]
    CA = core_inputs[0]["maskA"].shape[1]
    CB = core_inputs[0]["maskB"].shape[1]
    for ci in core_inputs:
        assert ci["idxA"].shape[1] == WA and ci["idxB"].shape[1] == WB
        assert ci["maskA"].shape[1] == CA and ci["maskB"].shape[1] == CB
    plan = core_inputs[0]["callplan"]

    nc = _build_nc(plan, WA, WB, CA, CB)

    betas = np.array([[beta1, beta2, beta3, 0.0]], dtype=np.float32)
    in_maps = []
    for st, ci in zip(per_core, core_inputs):
        xs = np.zeros((NSH, D), dtype=np.float32)
        pos = np.nonzero(st["perm"] >= 0)[0]
        xs[pos] = x[st["lo"] + st["perm"][pos]]
        in_maps.append(dict(x_shard=xs, idxA=ci["idxA"], idxB=ci["idxB"],
                            maskA=ci["maskA"], maskB=ci["maskB"], betas=betas))

    from concourse.bass_utils import run_bass_kernel_spmd

    try:
        r = run_bass_kernel_spmd(nc, in_maps, core_ids=list(range(NCORE)), trace=trace)
    except ModuleNotFoundError:
        r = run_bass_kernel_spmd(nc, in_maps, core_ids=list(range(NCORE)), trace=False)
    y = np.zeros((N, D), dtype=np.float32)
    for st, res in zip(per_core, r.results):
        pos = np.nonzero(st["perm"] >= 0)[0]
        y[st["lo"] + st["perm"][pos]] = np.asarray(res["y"])[pos]
    if _ret_info is not None:
        _ret_info["exec_time_ns"] = r.exec_time_ns
        _ret_info["results"] = r
    return y



# revision 29
# speedup vs baseline: 4.7907x; 4.7907x over previous
"""AGNN (3-layer) Trainium2 Bass kernel, 8-core SPMD.

Design:
  dst-node sharding (6250 real nodes/core, NSH=6272 padded positions).
  Gather table = raw node features (bf16, 256B rows) in compact global
  order, AllGathered per layer (12.85 MB). Per-edge dot AND source
  sumsq recomputed on DVE from gathered raw rows (interleaved P/Q
  shared halving tree), so no [xn|xu] 512B rows are needed.
  Self-loops handled analytically (score = beta exactly -> exp(beta)
  terms in numerator/denominator; no slots, no gather).
  Softmax without max-subtraction (|alpha| <= beta).
  Node layout: per-core snake sort by (kA, kB) degree classes over
  {1,2,3,4,6,8,12,16,24,32}; 128-node blocks take per-block max class,
  unified across cores (one NEFF). A/B source halves (4 cores each)
  keep gather indices within int16.
  y stays in SBUF between layers; repack perm->compact via dma_gather
  through DRAM at layer end feeds the next AllGather.
"""

import numpy as np
import sys, os
from contextlib import ExitStack

for _p in ("/opt/trn_rl_repo", "/root/.axon_site/_ro/trn_rl_repo"):
    if os.path.isdir(_p) and _p not in sys.path:
        try:
            import concourse  # noqa
            break
        except Exception:
            sys.path.insert(0, _p)

NCORE = 8
N = 50000
D = 128
NSH_REAL = 6250
NBLK = 49
NSH = NBLK * 128  # 6272
HALF = 4 * NSH  # 25088
NTAB = 8 * NSH  # 50176
KC = np.array([1, 2, 3, 4, 6, 8, 12, 16, 24, 32], dtype=np.int64)
CHMAX = 48  # max gather cols (128 slots each) per bucket per call
NEG = np.float32(-1e30)


def _classes(d):
    return KC[np.searchsorted(KC, np.maximum(d, 1))]


def _plan(edge_index):
    src = np.ascontiguousarray(edge_index[0]).astype(np.int64)
    dst = np.ascontiguousarray(edge_index[1]).astype(np.int64)
    core = dst // NSH_REAL
    loc = dst - core * NSH_REAL
    isA = src < HALF // NSH * NSH_REAL  # src < 25000

    key = core * NSH_REAL + loc
    dA = np.bincount(key[isA], minlength=N).reshape(NCORE, NSH_REAL)
    dB = np.bincount(key[~isA], minlength=N).reshape(NCORE, NSH_REAL)
    kA = _classes(dA)
    kB = _classes(dB)

    # snake order per core: kA asc, kB asc/desc alternating by kA class idx
    kci = np.zeros(int(KC[-1]) + 1, dtype=np.int64)
    kci[KC] = np.arange(len(KC))
    orders = np.empty((NCORE, NSH_REAL), dtype=np.int64)
    blocksA = np.zeros((NCORE, NBLK), dtype=np.int64)
    blocksB = np.zeros((NCORE, NBLK), dtype=np.int64)
    for c in range(NCORE):
        snake_kb = np.where((kci[kA[c]] % 2) == 1, -kB[c], kB[c])
        o = np.lexsort((snake_kb, kA[c]))
        orders[c] = o
        kAs = np.zeros(NSH, dtype=np.int64)
        kBs = np.zeros(NSH, dtype=np.int64)
        kAs[:NSH_REAL] = kA[c][o]
        kBs[:NSH_REAL] = kB[c][o]
        blocksA[c] = kAs.reshape(NBLK, 128).max(1)
        blocksB[c] = kBs.reshape(NBLK, 128).max(1)
    bA = blocksA.max(0)
    bB = blocksB.max(0)

    colbaseA = np.concatenate([[0], np.cumsum(bA)])
    colbaseB = np.concatenate([[0], np.cumsum(bB)])
    CA, CB = int(colbaseA[-1]), int(colbaseB[-1])
    calls = []
    b = 0
    while b < NBLK:
        e = b + 1
        while e < NBLK and bA[e] == bA[b] and bB[e] == bB[b]:
            e += 1
        kmax = max(bA[b], bB[b])
        npc = max(int(CHMAX // kmax), 1)
        for s in range(b, e, npc):
            nb = min(npc, e - s)
            calls.append(dict(
                ka=int(bA[b]), kb=int(bB[b]), b0=int(s), nb=int(nb),
                colA0=int(colbaseA[s]), colB0=int(colbaseB[s]),
                colsA=int(bA[b] * nb), colsB=int(bB[b] * nb),
            ))
        b = e

    SA, SB = CA * 128, CB * 128
    per_core = []
    for c in range(NCORE):
        o = orders[c]
        pos = np.empty(NSH_REAL, dtype=np.int64)
        pos[o] = np.arange(NSH_REAL)
        m = core == c
        e_src = src[m]
        e_loc = loc[m]
        e_isA = isA[m]
        q = e_loc * 2 + (~e_isA)
        s_idx = np.argsort(q, kind="stable")
        qs = q[s_idx]
        newgrp = np.ones(len(qs), dtype=bool)
        if len(qs) > 1:
            newgrp[1:] = qs[1:] != qs[:-1]
        starts = np.nonzero(newgrp)[0]
        grp_id = np.cumsum(newgrp) - 1
        rank_sorted = np.arange(len(qs)) - starts[grp_id]
        rank = np.empty(len(qs), dtype=np.int64)
        rank[s_idx] = rank_sorted
        p = pos[e_loc]
        blk = p // 128
        part = p - blk * 128
        src_core = e_src // NSH_REAL
        src_u = e_src - src_core * NSH_REAL
        tabrow = src_core * NSH + src_u
        idxA = np.zeros(SA, dtype=np.int64)
        idxB = np.zeros(SB, dtype=np.int64)
        maskA = np.full(SA, NEG, dtype=np.float32)
        maskB = np.full(SB, NEG, dtype=np.float32)
        a = e_isA
        iA = (colbaseA[blk[a]] + rank[a]) * 128 + part[a]
        idxA[iA] = tabrow[a]
        maskA[iA] = 0.0
        nb_ = ~a
        iB = (colbaseB[blk[nb_]] + rank[nb_]) * 128 + part[nb_]
        idxB[iB] = tabrow[nb_] - HALF
        maskB[iB] = 0.0
        idx0 = np.zeros(NSH, dtype=np.int64)
        idx0[:NSH_REAL] = o
        idxR = np.zeros(NSH, dtype=np.int64)
        idxR[:NSH_REAL] = pos

        def wrap16(v):
            return v.reshape(-1, 16).T.astype(np.int16)

        idxM = np.concatenate(
            [wrap16(idxA), wrap16(idxB), wrap16(idx0), wrap16(idxR)], axis=1)
        maskM = np.concatenate(
            [maskA.reshape(-1, 128).T, maskB.reshape(-1, 128).T],
            axis=1).astype(np.float32)
        per_core.append(dict(idxM=np.ascontiguousarray(idxM),
                             maskM=np.ascontiguousarray(maskM)))
    meta = dict(CA=CA, CB=CB, W=per_core[0]["idxM"].shape[1], calls=calls)
    return per_core, meta


def _build_nc(meta):
    import concourse.bass as bass
    import concourse.bacc as bacc
    import concourse.tile as tile
    from concourse import mybir, library_config

    f32, bf, i16 = mybir.dt.float32, mybir.dt.bfloat16, mybir.dt.int16
    Alu = mybir.AluOpType
    Act = mybir.ActivationFunctionType

    CA, CB, W = meta["CA"], meta["CB"], meta["W"]
    CM = CA + CB
    OFF0 = CM * 8
    OFFR = OFF0 + NSH // 16

    nc = bacc.Bacc("TRN2", target_bir_lowering=False, debug=False,
                   num_devices=NCORE, num_swdge_queues=1,
                   dynamic_dma_scratch_size=24576)
    x_tab_d = nc.dram_tensor("x_tab", [NSH, D], bf, kind="ExternalInput")
    idxM_d = nc.dram_tensor("idxM", [16, W], i16, kind="ExternalInput")
    maskM_d = nc.dram_tensor("maskM", [128, CM], f32, kind="ExternalInput")
    beta_d = nc.dram_tensor("betas", [1, 4], f32, kind="ExternalInput")
    y_d = nc.dram_tensor("y", [NSH, D], bf, kind="ExternalOutput")

    with ExitStack() as ctx:
        tc = ctx.enter_context(tile.TileContext(nc))
        res = ctx.enter_context(tc.tile_pool(name="res", bufs=1))
        dram = ctx.enter_context(tc.tile_pool(name="dram", bufs=1, space="DRAM"))
        gat = ctx.enter_context(tc.tile_pool(name="gat", bufs=2))
        tq = ctx.enter_context(tc.tile_pool(name="tq", bufs=2))
        sm = ctx.enter_context(tc.tile_pool(name="sm", bufs=2))
        vp = ctx.enter_context(tc.tile_pool(name="vp", bufs=2))
        nrm = ctx.enter_context(tc.tile_pool(name="nrm", bufs=1))
        ys = ctx.enter_context(tc.tile_pool(name="ys", bufs=2))

        idx_s = res.tile([128, W], i16)
        mask_s = res.tile([128, CM], f32)
        beta_s = res.tile([128, 4], f32)
        expb_s = res.tile([128, 4], f32)
        eps_s = res.tile([128, 1], f32)
        tab_shard = dram.tile([NSH, D], bf)
        tab_fulls = [dram.tile([NTAB, D], bf, addr_space="Shared",
                               name=f"tab_full{i}") for i in range(3)]
        y_dram = dram.tile([NSH, D], bf)

        nc.gpsimd.load_library(library_config.mlp)
        for b in range(8):
            nc.sync.dma_start(out=idx_s[16 * b : 16 * (b + 1), :], in_=idxM_d[:])
        nc.sync.dma_start(out=mask_s[:], in_=maskM_d[:])
        bap = beta_d[:]
        nc.gpsimd.dma_start(
            out=beta_s[:],
            in_=bass.AP(tensor=bap.tensor, offset=bap.offset, ap=[[0, 128], [1, 4]]),
        )
        nc.vector.memset(eps_s[:], 1e-20)
        nc.scalar.activation(out=expb_s[:], in_=beta_s[:], func=Act.Exp)

        _regcache = {}

        def nreg(v):
            if v not in _regcache:
                _regcache[v] = nc.gpsimd.to_reg(v)
            return _regcache[v]

        GCH = 8  # gather chunk: 8 cols = 1024 idxs (ring holds 2048 descs)

        def gather_chunked(out3, in_ap, colbase, cols):
            # out3: [128, cols, D] SBUF view; idx cols start at colbase*8
            for c0 in range(0, cols, GCH):
                cw = min(GCH, cols - c0)
                nc.gpsimd.dma_gather(
                    out_ap=out3[:, c0 : c0 + cw, :],
                    in_ap=in_ap,
                    idxs_ap=idx_s[:, (colbase + c0) * 8 : (colbase + c0 + cw) * 8],
                    num_idxs=cw * 128,
                    num_idxs_reg=nreg(cw * 128),
                    elem_size=D, queue_num=0)

        def bcast_mid(ap3, k):
            a = ap3.ap
            return bass.AP(tensor=ap3.tensor, offset=ap3.offset,
                           ap=[a[0], a[1], [0, k], a[2]])

        def bcast_last(ap2, k):
            a = ap2.ap
            return bass.AP(tensor=ap2.tensor, offset=ap2.offset,
                           ap=[a[0], a[1], [0, k]])

        def ktree(V4, k):
            # in-place sum over the k axis of V4=[128, nb, k, D] -> [..., 0:1, :]
            while k > 1:
                h = k // 2
                nc.vector.tensor_add(out=V4[:, :, 0:h, :], in0=V4[:, :, 0:h, :],
                                     in1=V4[:, :, h : 2 * h, :])
                if k - 2 * h:
                    nc.vector.tensor_add(
                        out=V4[:, :, 0:1, :], in0=V4[:, :, 0:1, :],
                        in1=V4[:, :, 2 * h : 2 * h + 1, :])
                k = h
            return V4[:, :, 0:1, :]

        for layer in range(3):
            if layer == 0:
                xtc = nrm.tile([128, NBLK, D], bf, tag="ycomp", name="xtc")
                nc.sync.dma_start(
                    out=xtc[:],
                    in_=x_tab_d[:].rearrange("(b m) d -> m b d", m=128))
                tsv = tab_shard[:].rearrange("(b m) d -> m b d", m=128)
                nc.sync.dma_start(out=tsv, in_=xtc[:])
                xsb = ys.tile([128, NBLK, D], bf, tag="y", name="xsb0")
                gather_chunked(xsb[:], x_tab_d[:], OFF0 // 8, NBLK)

            # ---- normalize own shard (dst side) ----
            junk = nrm.tile([128, NBLK, D], bf, tag="ycomp", name="junk")
            nc.vector.tensor_mul(out=junk[:], in0=xsb[:], in1=xsb[:])
            h = D // 2
            while h >= 16:
                nc.vector.tensor_add(out=junk[:, :, 0:h], in0=junk[:, :, 0:h],
                                     in1=junk[:, :, h : 2 * h])
                h //= 2
            nf = nrm.tile([128, NBLK], f32, tag="nf", name="nf")
            nc.vector.tensor_reduce(out=nf[:], in_=junk[:, :, 0:16],
                                    axis=mybir.AxisListType.X, op=Alu.add)
            rstd = nrm.tile([128, NBLK], f32, tag="rstd")
            nc.scalar.activation(
                out=rstd[:], in_=nf[:],
                func=Act.Sqrt, bias=eps_s[:], scale=1.0)
            nc.vector.reciprocal(out=rstd[:], in_=rstd[:])
            xn = nrm.tile([128, NBLK, D], bf, tag="xn")
            nc.vector.tensor_tensor(out=xn[:], in0=xsb[:],
                                    in1=bcast_last(rstd[:], D), op=Alu.mult)

            # ---- AllGather compact raw table ----
            tab_full = tab_fulls[layer]
            nc.gpsimd.collective_compute(
                "AllGather", Alu.bypass, replica_groups=[list(range(NCORE))],
                ins=[tab_shard[:]], outs=[tab_full[:]],
            )

            y_new = ys.tile([128, NBLK, D], bf, tag="y", name="ynew")
            ZallA = sm.tile([128, NBLK], f32, tag="ZallA", name="ZallA")
            ZallB = sm.tile([128, NBLK], f32, tag="ZallB", name="ZallB")

            for call in meta["calls"]:
                ka, kb, b0, nb = call["ka"], call["kb"], call["b0"], call["nb"]

                def phase1(kx, colsX, col0, base, gtag):
                    # gather + dot/sumsq products + shared halving tree
                    t = gat.tile([128, CHMAX, D], bf, tag="t" + gtag,
                                 name="t")[:, 0:colsX, :]
                    gather_chunked(t, tab_full[base : base + HALF, :],
                                   col0, colsX)
                    T = tq.tile([128, CHMAX, 2, D], bf, tag="T",
                                name="T")[:, 0:colsX, :, :]
                    xnsl = xn[:, b0 : b0 + nb, :]
                    nc.vector.tensor_tensor(
                        out=T[:, :, 0, :].rearrange("p (b k) d -> p b k d", k=kx),
                        in0=t.rearrange("p (b k) d -> p b k d", k=kx),
                        in1=bcast_mid(xnsl, kx), op=Alu.mult)
                    nc.vector.tensor_mul(out=T[:, :, 1, :], in0=t, in1=t)
                    h = D // 2
                    while h >= 16:
                        nc.vector.tensor_add(out=T[:, :, :, 0:h],
                                             in0=T[:, :, :, 0:h],
                                             in1=T[:, :, :, h : 2 * h])
                        h //= 2
                    qf = tq.tile([128, CHMAX, 2], f32, tag="qf",
                                 name="qf")[:, 0:colsX, :]
                    nc.vector.tensor_reduce(out=qf, in_=T[:, :, :, 0:16],
                                            axis=mybir.AxisListType.X,
                                            op=Alu.add)
                    return t, qf

                def sqrtQ(qf, colsX, gtag):
                    Q = qf[:, :, 1:2].rearrange("p c one -> p (c one)")
                    R = sm.tile([128, CHMAX], f32, tag="R" + gtag,
                                name="R")[:, 0:colsX]
                    nc.scalar.activation(out=R, in_=Q, func=Act.Sqrt,
                                         bias=eps_s[:], scale=1.0)
                    return R

                def alphaE(qf, R, colsX, col0, gtag):
                    P = qf[:, :, 0:1].rearrange("p c one -> p (c one)")
                    nc.vector.reciprocal(out=R, in_=R)
                    AL = sm.tile([128, CHMAX], f32, tag="A" + gtag,
                                 name="AL")[:, 0:colsX]
                    nc.vector.tensor_mul(out=AL, in0=P, in1=R)
                    nc.vector.scalar_tensor_tensor(
                        out=AL, in0=AL, scalar=beta_s[:, layer : layer + 1],
                        in1=mask_s[:, col0 : col0 + colsX],
                        op0=Alu.mult, op1=Alu.add)
                    return AL

                def expE(AL, colsX, gtag):
                    # E duplicated x2 so downstream broadcasts keep a packed
                    # last dim (2x DVE mode)
                    E2 = sm.tile([128, CHMAX, 2], bf, tag="E" + gtag,
                                 name="E2")[:, 0:colsX, :]
                    nc.scalar.activation(
                        out=E2, in_=bass.AP(tensor=AL.tensor, offset=AL.offset,
                                            ap=[AL.ap[0], AL.ap[1], [0, 2]]),
                        func=Act.Exp)
                    return E2

                def vphase(t, E2, kx, colsX, Zall):
                    nc.vector.tensor_reduce(
                        out=Zall[:, b0 : b0 + nb],
                        in_=E2[:, :, 0].rearrange("p (b k) -> p b k", k=kx),
                        axis=mybir.AxisListType.X, op=Alu.add)
                    V = vp.tile([128, CHMAX, D], bf, tag="V",
                                name="V")[:, 0:colsX, :]
                    e_ap = bass.AP(tensor=E2.tensor, offset=E2.offset,
                                   ap=[E2.ap[0], E2.ap[1], [0, D // 2],
                                       E2.ap[2]])
                    nc.vector.tensor_tensor(
                        out=V.rearrange("p c (e two) -> p c e two", two=2),
                        in0=t.rearrange("p c (e two) -> p c e two", two=2),
                        in1=e_ap, op=Alu.mult)
                    Vs = ktree(V.rearrange("p (b k) d -> p b k d", k=kx), kx)
                    return Vs

                cA = (call["colsA"], call["colA0"], 0, "gA")
                cB = (call["colsB"], CA + call["colB0"], HALF, "gB")
                tA, qfA = phase1(ka, *cA[0:1], cA[1], cA[2], cA[3])
                tB, qfB = phase1(kb, *cB[0:1], cB[1], cB[2], cB[3])
                RA = sqrtQ(qfA, cA[0], "gA")
                RB = sqrtQ(qfB, cB[0], "gB")
                ALA = alphaE(qfA, RA, cA[0], cA[1], "gA")
                ALB = alphaE(qfB, RB, cB[0], cB[1], "gB")
                EA2 = expE(ALA, cA[0], "gA")
                EB2 = expE(ALB, cB[0], "gB")
                VA = vphase(tA, EA2, ka, cA[0], ZallA)
                VB = vphase(tB, EB2, kb, cB[0], ZallB)

                ysl = y_new[:, b0 : b0 + nb, :]
                nc.vector.tensor_add(
                    out=ysl, in0=VA.rearrange("p b one d -> p (b one) d"),
                    in1=VB.rearrange("p b one d -> p (b one) d"))
                nc.vector.scalar_tensor_tensor(
                    out=ysl, in0=xsb[:, b0 : b0 + nb, :],
                    scalar=expb_s[:, layer : layer + 1], in1=ysl,
                    op0=Alu.mult, op1=Alu.add)

            # ---- batched denominator ----
            Zt = sm.tile([128, NBLK], f32, tag="Zt", name="Zt")
            ebl = expb_s[:, layer : layer + 1]
            ebl_b = bass.AP(tensor=ebl.tensor, offset=ebl.offset,
                            ap=[ebl.ap[0], [0, NBLK]])
            nc.vector.tensor_add(out=Zt[:], in0=ZallA[:], in1=ZallB[:])
            nc.vector.tensor_tensor(out=Zt[:], in0=Zt[:], in1=ebl_b, op=Alu.add)
            nc.vector.reciprocal(out=Zt[:], in_=Zt[:])
            nc.vector.tensor_tensor(out=y_new[:], in0=y_new[:],
                                    in1=bcast_last(Zt[:], D), op=Alu.mult)

            # ---- repack perm -> compact; feed next AG / output ----
            ydv = y_dram[:].rearrange("(b m) d -> m b d", m=128)
            nc.sync.dma_start(out=ydv, in_=y_new[:])
            ycomp = nrm.tile([128, NBLK, D], bf, tag="ycomp", name="ycomp")
            gather_chunked(ycomp[:], y_dram[:], OFFR // 8, NBLK)
            if layer < 2:
                tsv = tab_shard[:].rearrange("(b m) d -> m b d", m=128)
                nc.sync.dma_start(out=tsv, in_=ycomp[:])
                xsb = y_new
            else:
                yov = y_d[:].rearrange("(b m) d -> m b d", m=128)
                nc.sync.dma_start(out=yov, in_=ycomp[:])
    nc.compile()
    return nc


_CACHE = {}


def prepare(x, edge_index, beta1, beta2, beta3):
    import ml_dtypes
    edge_index = np.asarray(edge_index)
    key = hash(edge_index.tobytes())
    if key not in _CACHE:
        per_core, meta = _plan(edge_index)
        nc = _build_nc(meta)
        _CACHE[key] = (per_core, meta, nc)
    per_core, meta, nc = _CACHE[key]

    x = np.asarray(x, dtype=np.float32)
    betas = np.array([[beta1, beta2, beta3, 0.0]], dtype=np.float32)
    in_maps = []
    for c, pc in enumerate(per_core):
        xt = np.empty((NSH, D), dtype=np.float32)
        xt[:NSH_REAL] = x[c * NSH_REAL : (c + 1) * NSH_REAL]
        xt[NSH_REAL:] = x[c * NSH_REAL]
        in_maps.append(dict(x_tab=xt.astype(ml_dtypes.bfloat16),
                            idxM=pc["idxM"], maskM=pc["maskM"], betas=betas))

    def unshard(ys_list):
        out = np.empty((N, D), dtype=np.float32)
        for c, yv in enumerate(ys_list):
            out[c * NSH_REAL : (c + 1) * NSH_REAL] = np.asarray(
                yv[:NSH_REAL]).astype(np.float32)
        return out

    return nc, in_maps, unshard


def kernel(x, edge_index, beta1, beta2, beta3, trace=False, _ret_info=None):
    nc, in_maps, unshard = prepare(x, edge_index, beta1, beta2, beta3)
    from concourse.bass_utils import run_bass_kernel_spmd

    try:
        r = run_bass_kernel_spmd(nc, in_maps, core_ids=list(range(NCORE)),
                                 trace=trace)
    except ModuleNotFoundError:
        r = run_bass_kernel_spmd(nc, in_maps, core_ids=list(range(NCORE)),
                                 trace=False)
    y = unshard([res["y"] for res in r.results])
    if _ret_info is not None:
        _ret_info["exec_time_ns"] = r.exec_time_ns
        _ret_info["results"] = r
    return y
